# revision 1
# baseline (speedup 1.0000x reference)
"""Trainium2 Bass kernel for nn_Addparam_25701084299720 (retrieval_knn).

Computes, for N=4096 query points against V=16384 voxels:
  - 8-NN of each query (exact, matching fp32 diff-formulation ordering)
  - mean normal of the 8-NN, cosine-threshold mask vs all voxel normals
  - score_sum = sum_v mask * score_v/d_a * exp(-d_b*dist), score_num = sum mask
  - field = score_sum/max(score_num,1) masked by score_num!=0

Sharding: data-parallel over queries, 512 per core across 8 cores.
Voxel tables replicated.

Per-core algorithm (per 128-query tile):
  A. mmA: psum = 2 x.p - |p|^2 - |x|^2 = -(dist^2), bf16x3-split matmul
     (K=24, ~1e-7 exact). Per chunk: segment-max (16-voxel segments) ->
     segsel[q, 1024] candidate ranking; ACT sqrt psum->D chunk (fp16);
     ACT exp -> E[q, v] (bf16), later scaled in-place by score/d_a (gpsimd).
  B. top-8 segments per half of segsel (DVE max/max_index, 512-wide) ->
     16 segments -> indirect-DMA gather of packed (p, n) segment blocks ->
     exact fp32 diff-formulation rescore of 256 candidates (matches the
     reference's rounding exactly) -> top-8 -> xn = sum of their normals.
  C. mmC: margin = xn.nv - 0.8|xn||nv| (bf16x2/3, K=21). Fused DVE
     scalar_tensor_tensor: ss += sum((margin>0) * E*score'); tensor_scalar:
     cnt += sum(margin>0).
  D. field = ss/max(cnt,1) * (cnt>0).
"""
import sys

sys.path.insert(0, "/opt/trn_rl_repo")

import numpy as np
import ml_dtypes

N_CORES = 8
N = 4096
V = 16384
NQ = N // N_CORES          # 512 queries per core
P = 128                    # partitions
NT = NQ // P               # 4 query tiles per core
VCH = 512                  # voxel chunk (free dim per matmul)
NCH = V // VCH             # 32 chunks
SEGW = 16                  # voxels per segment
NSEG = V // SEGW           # 1024 segments
SPC = VCH // SEGW          # 32 segments per chunk
NCSEG = 16                 # candidate segments (8 per half)
NCAND = NCSEG * SEGW       # 256 candidate voxels
SUBR = 16                  # psum-pair sub-round for ACT batching

BF = ml_dtypes.bfloat16
F32 = np.float32

_prog_cache = {}


def _split3(x):
    """bf16x3 decomposition of fp32 array -> (hi, mid, lo) as float32."""
    hi = x.astype(BF).astype(F32)
    mid = (x - hi).astype(BF).astype(F32)
    lo = (x - hi - mid).astype(BF).astype(F32)
    return hi, mid, lo


def _build_program(neg_db: float, hw: bool = True):
    import concourse.bass as bass
    import concourse.mybir as mybir
    from concourse.tile import TileContext

    nc = bass.Bass()
    dt = mybir.dt
    AF = mybir.ActivationFunctionType
    OP = mybir.AluOpType

    lA_d = nc.declare_dram_parameter("lA", [24, NQ], dt.bfloat16, isOutput=False)
    rA_d = nc.declare_dram_parameter("rA", [24, V], dt.bfloat16, isOutput=False)
    rC_d = nc.declare_dram_parameter("rC", [21, V], dt.bfloat16, isOutput=False)
    pk_d = nc.declare_dram_parameter("pk", [NSEG, SEGW * 8], dt.float32,
                                     isOutput=False)
    scp_d = nc.declare_dram_parameter("scp", [V], dt.bfloat16, isOutput=False)
    xq_d = nc.declare_dram_parameter("xq", [NQ, 3], dt.float32, isOutput=False)
    idm_d = nc.declare_dram_parameter("idm", [P, P], dt.float32, isOutput=False)
    of_d = nc.declare_dram_parameter("of", [NQ], dt.float32, isOutput=True)
    on_d = nc.declare_dram_parameter("on", [NQ], dt.float32, isOutput=True)

    ts = bass.ts
    from concourse.tile_rust import add_dep_helper

    def act(*args, **kwargs):
        return nc.scalar.activation(*args, **kwargs)

    with TileContext(nc) as tc:
        with (
            tc.tile_pool(name="const", bufs=1) as constp,
            tc.tile_pool(name="bigd", bufs=2) as bigp,
            tc.tile_pool(name="small1", bufs=1) as smp1,
            tc.tile_pool(name="chunk", bufs=2) as chp,
            tc.tile_pool(name="dch", bufs=SUBR + 1) as dchp,
            tc.tile_pool(name="small", bufs=2) as smp,
            tc.tile_pool(name="psA", bufs=2, space="PSUM") as psA,
            tc.tile_pool(name="psC", bufs=2, space="PSUM") as psC,
            tc.tile_pool(name="psT", bufs=2, space="PSUM") as psT,
        ):
            lA = constp.tile([24, NQ], dt.bfloat16)
            prc = constp.tile([53, V], dt.bfloat16)
            rA = prc[0:24, :]
            rC = prc[32:53, :]
            scbc = constp.tile([P, V], dt.bfloat16)
            ident = constp.tile([P, P], dt.float32)
            eps4 = constp.tile([P, 1], dt.float32)
            nc.vector.memset(eps4[:], 1e-4)
            nc.sync.dma_start(lA[:], lA_d[:])
            Q4 = V // 4
            nc.sync.dma_start(prc[0:24, 0:Q4], rA_d[:, 0:Q4])
            nc.sync.dma_start(prc[0:24, Q4:2 * Q4], rA_d[:, Q4:2 * Q4])
            nc.scalar.dma_start(prc[0:24, 2 * Q4:3 * Q4], rA_d[:, 2 * Q4:3 * Q4])
            nc.scalar.dma_start(prc[0:24, 3 * Q4:V], rA_d[:, 3 * Q4:V])
            nc.gpsimd.dma_start(prc[32:53, :], rC_d[:])
            nc.gpsimd.dma_start(
                scbc[:],
                scp_d[:].rearrange("(o v) -> o v", o=1).to_broadcast([P, V]),
            )
            nc.gpsimd.dma_start(ident[:], idm_d[:])

            # ---------------- pipelined phase emission ----------------
            def emit_A(i, cprev):
                """Phase A of tile i, with tile i-1's 32 C-chunks interleaved
                (2 per psum-pair) so PE/DVE/Pool keep working while ACT runs
                the sqrt batch."""
                E0 = bigp.tile([P, V // 2], dt.bfloat16, tag="E0")
                E1 = bigp.tile([P, V // 2], dt.bfloat16, tag="E1")
                Eh = [E0, E1]
                segsel = smp.tile([P, NSEG], dt.float16, tag="segsel")
                xqt = smp.tile([P, 3], dt.float32, tag="xqt")
                nc.sync.dma_start(xqt[:], xq_d[ts(i, P), :])
                dchs = []
                sqrt_insts = []
                for u in range(NCH // 2):
                    pA = psA.tile([P, 2 * VCH], dt.float32)
                    for half in range(2):
                        nc.tensor.matmul(
                            pA[:, ts(half, VCH)], lA[:, ts(i, P)],
                            rA[:, ts(2 * u + half, VCH)],
                            start=True, stop=True,
                        )
                    Dch = dchp.tile([P, 2 * VCH], dt.float16, tag="Dch")
                    si = act(
                        Dch[:], pA[:], AF.Sqrt,
                        bias=eps4[:, 0:1], scale=-1.0,
                    )
                    sqrt_insts.append(si)
                    nc.vector.tensor_reduce(
                        segsel[:, ts(u, 2 * SPC)],
                        Dch[:].rearrange("p (s w) -> p s w", w=SEGW),
                        axis=mybir.AxisListType.X, op=OP.min, negate=True,
                    )
                    dchs.append(Dch)
                    if cprev is not None:
                        if u == 2:
                            emit_finishB(cprev)
                        if u >= 3:
                            j0 = 2 * (u - 3)
                            emit_C_chunks(cprev, j0, j0 + 2)
                if cprev is not None:
                    emit_C_chunks(cprev, 26, NCH)
                for k in range(NCH // 2):
                    e_half = Eh[(2 * k) // (NCH // 2)]
                    off = (2 * k * VCH) % (V // 2)
                    ei = act(
                        e_half[:, off:off + 2 * VCH], dchs[k][:],
                        AF.Exp, scale=neg_db,
                    )
                    if k == 0:
                        add_dep_helper(sqrt_insts[-1].ins, ei.ins, sync=True,
                                       reason="exp run after sqrt run")
                # E <- E * score', sliced so it pipelines under the exp run
                NSL = 8
                SL = V // NSL
                for q in range(NSL):
                    h = q // (NSL // 2)
                    off = (q * SL) % (V // 2)
                    nc.gpsimd.tensor_tensor(
                        Eh[h][:, off:off + SL], Eh[h][:, off:off + SL],
                        scbc[:, q * SL:(q + 1) * SL], OP.mult
                    )
                return {"i": i, "Eh": Eh, "segsel": segsel, "xqt": xqt}

            def emit_B(a):
                segsel, xqt = a["segsel"], a["xqt"]
                m8s = smp.tile([P, NCSEG], dt.float16, tag="m8s")
                sidx = smp.tile([P, NCSEG], dt.uint32, tag="sidx")
                HS = NSEG // 2
                nc.vector.max(m8s[:, 0:8], segsel[:, 0:HS])
                nc.vector.max(m8s[:, 8:16], segsel[:, HS:NSEG])
                nc.vector.max_index(sidx[:, 0:8], m8s[:, 0:8], segsel[:, 0:HS])
                nc.vector.max_index(sidx[:, 8:16], m8s[:, 8:16],
                                    segsel[:, HS:NSEG])
                nc.vector.tensor_scalar(
                    sidx[:, 8:16], sidx[:, 8:16], HS, None, OP.add
                )
                pkg = smp1.tile([P, NCSEG, SEGW * 8], dt.float32, tag="pkg")
                # hardware indirect DMA consumes ONE index per partition
                # (tile_scatter_add pattern): one gather per candidate segment
                for g in range(NCSEG):
                    nc.gpsimd.indirect_dma_start(
                        out=pkg[:, g, :], out_offset=None,
                        in_=pk_d[:],
                        in_offset=bass.IndirectOffsetOnAxis(
                            ap=sidx[:, g:g + 1], axis=0),
                    )
                pkv = pkg[:].rearrange("p s (w c) -> p s w c", c=8)
                sq0 = smp1.tile([P, NCAND], dt.float32, tag="sq0")
                sq1 = smp1.tile([P, NCAND], dt.float32, tag="sq1")
                sq2 = smp1.tile([P, NCAND], dt.float32, tag="sq2")
                df0 = smp1.tile([P, NCAND], dt.float32, tag="df0")
                df1 = smp1.tile([P, NCAND], dt.float32, tag="df1")
                df2 = smp1.tile([P, NCAND], dt.float32, tag="df2")
                for c in range(3):
                    df = (df0, df1, df2)[c]
                    sq = (sq0, sq1, sq2)[c]
                    # fl(p - x) then fl(square): same rounding as reference
                    nc.vector.tensor_scalar(
                        df[:], pkv[:, :, :, c], xqt[:, c:c + 1], None,
                        OP.subtract,
                    )
                    nc.gpsimd.tensor_tensor(sq[:], df[:], df[:], OP.mult)
                exd2 = smp1.tile([P, NCAND], dt.float32, tag="exd2")
                nc.gpsimd.tensor_tensor(exd2[:], sq0[:], sq1[:], OP.add)
                nc.gpsimd.tensor_tensor(exd2[:], exd2[:], sq2[:], OP.add)
                negk = smp1.tile([P, NCAND], dt.float32, tag="negk")
                nc.vector.tensor_scalar(negk[:], exd2[:], -1.0, None, OP.mult)
                m8x = smp.tile([P, 8], dt.float32, tag="m8x")
                nc.vector.max(m8x[:], negk[:])
                selx = smp1.tile([P, NCAND], dt.float32, tag="selx")
                nc.vector.tensor_scalar(
                    selx[:], negk[:], m8x[:, 7:8], None, OP.is_ge
                )
                xa4 = smp.tile([P, 4], dt.float32, tag="xa4")
                trc = smp1.tile([P, NCAND], dt.float32, tag="trc")
                for c in range(3):
                    nc.vector.tensor_tensor(
                        trc[:], selx[:], pkv[:, :, :, 3 + c], OP.mult
                    )
                    nc.vector.reduce_sum(
                        xa4[:, c:c + 1], trc[:], axis=mybir.AxisListType.X
                    )
                a2 = smp.tile([P, 1], dt.float32, tag="a2")
                tr3 = smp.tile([P, 3], dt.float32, tag="tr3")
                nc.vector.tensor_tensor(
                    tr3[:], xa4[:, 0:3], xa4[:, 0:3], OP.mult
                )
                nc.vector.reduce_sum(a2[:], tr3[:], axis=mybir.AxisListType.X)


                # lCt [128,21] f32 (free-dim writes only), then PE-transpose.
                # rows k: 0-2 xnh, 3-5 xnh, 6-8 xnm, 9-11 xnh, 12-14 xnl,
                #         15-17 xnm, 18 ch, 19 ch, 20 cm
                lCt = smp.tile([P, 21], dt.float32, tag="lCt")
                bf3 = smp.tile([P, 3], dt.bfloat16, tag="bf3")
                t32 = smp.tile([P, 3], dt.float32, tag="t32")
                rem = smp.tile([P, 3], dt.float32, tag="rem")
                crow = smp.tile([P, 1], dt.float32, tag="crow")
                nc.vector.tensor_copy(bf3[:], xa4[:, 0:3])
                nc.vector.tensor_copy(t32[:], bf3[:])
                nc.vector.tensor_copy(lCt[:, 0:3], t32[:])
                nc.vector.tensor_copy(lCt[:, 3:6], t32[:])
                nc.vector.tensor_copy(lCt[:, 9:12], t32[:])
                nc.vector.tensor_tensor(rem[:], xa4[:, 0:3], t32[:], OP.subtract)
                nc.vector.tensor_copy(bf3[:], rem[:])
                nc.vector.tensor_copy(t32[:], bf3[:])
                nc.vector.tensor_copy(lCt[:, 6:9], t32[:])
                nc.vector.tensor_copy(lCt[:, 15:18], t32[:])
                nc.vector.tensor_tensor(rem[:], rem[:], t32[:], OP.subtract)
                nc.vector.tensor_copy(bf3[:], rem[:])
                nc.vector.tensor_copy(lCt[:, 12:15], bf3[:])
                return {"i": a["i"], "Eh": a["Eh"], "lCt": lCt, "a2": a2,
                        "xa4": xa4, "bf3": bf3, "t32": t32, "rem": rem,
                        "crow": crow}

            def emit_finishB(b):
                xa4, bf3, t32 = b["xa4"], b["bf3"], b["t32"]
                rem, crow, lCt = b["rem"], b["crow"], b["lCt"]
                act(xa4[:, 3:4], b["a2"][:], AF.Sqrt)
                nc.vector.tensor_scalar(
                    crow[:], xa4[:, 3:4], -0.8, None, OP.mult
                )
                nc.vector.tensor_copy(bf3[:, 0:1], crow[:])
                nc.vector.tensor_copy(t32[:, 0:1], bf3[:, 0:1])
                nc.vector.tensor_copy(lCt[:, 18:19], t32[:, 0:1])
                nc.vector.tensor_copy(lCt[:, 19:20], t32[:, 0:1])
                nc.vector.tensor_tensor(
                    rem[:, 0:1], crow[:], t32[:, 0:1], OP.subtract
                )
                nc.vector.tensor_copy(bf3[:, 0:1], rem[:, 0:1])
                nc.vector.tensor_copy(lCt[:, 20:21], bf3[:, 0:1])
                pTC = psT.tile([21, P], dt.float32)
                nc.tensor.transpose(pTC[:], lCt[:], ident[:])
                # lC lives at base partition 32 to match rC (packed in prc)
                lCp = smp.tile([53, P], dt.bfloat16, tag="lCp")
                lC = lCp[32:53, :]
                nc.vector.tensor_copy(lC, pTC[:])
                b["lC"] = lC
                ss32 = smp.tile([P, NCH], dt.float32, tag="ss32")
                cnt32 = smp.tile([P, NCH], dt.float32, tag="cnt32")
                b["ss32"] = ss32
                b["cnt32"] = cnt32

            def emit_C_chunks(b, j0, j1, tail=False):
                lC, Eh, ss32, cnt32 = b["lC"], b["Eh"], b["ss32"], b["cnt32"]
                for j in range(j0, j1):
                    pC = psC.tile([P, VCH], dt.float32)
                    nc.tensor.matmul(
                        pC[:], lC, rC[:, ts(j, VCH)], start=True, stop=True
                    )
                    e_half = Eh[j // (NCH // 2)]
                    off = (j * VCH) % (V // 2)
                    if tail:
                        # ACT is idle in the tail: sign(margin) = +-1, then
                        # ss_j = sum((sign+1)*E) = 2*sum(msk*E),
                        # cnt_j = sum(sign+1)/... = 2*cnt; halved in emit_D.
                        sgn = chp.tile([P, VCH], dt.bfloat16, tag="sgn")
                        act(sgn[:], pC[:], AF.Sign)
                        trb = chp.tile([P, VCH], dt.bfloat16, tag="trb")
                        nc.vector.scalar_tensor_tensor(
                            out=trb[:], in0=sgn[:], scalar=1.0,
                            in1=e_half[:, off:off + VCH],
                            op0=OP.add, op1=OP.mult,
                            accum_out=ss32[:, j:j + 1],
                        )
                        trb2 = chp.tile([P, VCH], dt.bfloat16, tag="trb2")
                        nc.vector.tensor_scalar(
                            trb2[:], pC[:], 0.0, None, OP.is_gt, OP.add,
                            accum_out=cnt32[:, j:j + 1],
                        )
                    else:
                        msk = chp.tile([P, VCH], dt.bfloat16, tag="msk")
                        nc.vector.tensor_scalar(
                            msk[:], pC[:], 0.0, None, OP.is_gt, OP.add,
                            accum_out=cnt32[:, j:j + 1],
                        )
                        ms = chp.tile([P, VCH], dt.bfloat16, tag="ms")
                        nc.gpsimd.tensor_tensor(
                            ms[:], msk[:], e_half[:, off:off + VCH], OP.mult
                        )
                        trb = chp.tile([P, VCH], dt.bfloat16, tag="trb")
                        nc.vector.tensor_scalar(
                            trb[:], ms[:], 0.0, None, OP.add, OP.add,
                            accum_out=ss32[:, j:j + 1],
                        )

            def emit_D(b, tail=False):
                i, ss32, cnt32 = b["i"], b["ss32"], b["cnt32"]
                sst = smp.tile([P, 1], dt.float32, tag="sst")
                cntt = smp.tile([P, 1], dt.float32, tag="cntt")
                nc.vector.reduce_sum(sst[:], ss32[:], axis=mybir.AxisListType.X)
                nc.vector.reduce_sum(cntt[:], cnt32[:],
                                     axis=mybir.AxisListType.X)
                if tail:
                    nc.vector.tensor_scalar(sst[:], sst[:], 0.5, None, OP.mult)
                nz = smp.tile([P, 1], dt.float32, tag="nz")
                nc.vector.tensor_scalar(nz[:], cntt[:], 0.5, None, OP.is_gt)
                cc = smp.tile([P, 1], dt.float32, tag="cc")
                nc.vector.tensor_scalar(cc[:], cntt[:], 1.0, None, OP.max)
                rec = smp.tile([P, 1], dt.float32, tag="rec")
                nc.vector.reciprocal(rec[:], cc[:])
                fld = smp.tile([P, 1], dt.float32, tag="fld")
                nc.vector.tensor_tensor(fld[:], sst[:], rec[:], OP.mult)
                nc.vector.tensor_tensor(fld[:], fld[:], nz[:], OP.mult)
                nc.sync.dma_start(of_d[ts(i, P)], fld[:])
                nc.sync.dma_start(on_d[ts(i, P)], nz[:])

            prev = None
            for i in range(NT):
                a = emit_A(i, prev)
                if prev is not None:
                    emit_D(prev)
                prev = emit_B(a)
            emit_finishB(prev)
            emit_C_chunks(prev, 0, NCH)
            emit_D(prev)

    if hw:
        _split_multiwaits(nc)
    return nc


def _split_multiwaits(nc):
    """This toolchain's walrus accepts at most ONE sync wait per
    instruction (setupSyncWait<...> hard-errors otherwise). Tile attaches
    all required waits to the consuming instruction, so split every
    extra wait into a standalone EventSemaphore on the same engine queue
    right before the instruction (the raw-Bass wait_ge pattern)."""
    import concourse.mybir as mybir

    n = 0
    for bb in nc.main_func.blocks:
        insts = bb.instructions
        out = []
        for inst in insts:
            si = inst.sync_info
            if si is not None and len(si.on_wait) > 1:
                waits = list(si.on_wait)
                for w in waits[:-1]:
                    ev = mybir.InstEventSemaphore(name=f"W-split-{n}")
                    n += 1
                    ev.engine = inst.engine
                    ev.debug = inst.debug
                    ev.sync_info = mybir.SyncInfo(on_wait=[w], on_update=[])
                    out.append(ev)
                inst.sync_info = mybir.SyncInfo(
                    on_wait=[waits[-1]], on_update=list(si.on_update)
                )
            out.append(inst)
        bb.instructions = out


def _prep_inputs(x_world, voxel_point, voxel_normal, score, d_a, d_b):
    """Host-side prep: per-core in_maps for the SPMD program."""
    x = np.ascontiguousarray(x_world[:, 0, :], dtype=F32)          # [N,3]
    p = np.ascontiguousarray(voxel_point[0, :, :3], dtype=F32)     # [V,3]
    nrm = np.ascontiguousarray(voxel_normal, dtype=F32)            # [V,3]
    sc = np.asarray(score, dtype=F32)
    da = float(np.asarray(d_a).reshape(-1)[0])
    db = float(np.asarray(d_b).reshape(-1)[0])

    # rA rows 0-17: p-side of products (xh,ph),(xh,pm),(xm,ph),(xh,pl),
    # (xl,ph),(xm,pm) [3 coord rows each]; 18-20: -p2 h/m/l (lhsT ones);
    # 21-23: ones (lhsT -x2 h/m/l).  psum = 2 x.p - p2 - x2 = -(dist^2)
    ph, pm, pl = _split3(p)
    p2 = (p * p).sum(1, dtype=F32).astype(F32)
    np2h, np2m, np2l = [(-t).astype(F32) for t in _split3(p2)]
    ones_v = np.ones((1, V), F32)
    rA = np.concatenate(
        [ph.T, pm.T, ph.T, pl.T, ph.T, pm.T,
         np2h[None], np2m[None], np2l[None], ones_v, ones_v, ones_v], axis=0,
    ).astype(BF)                                                    # [24,V]

    b = np.sqrt((nrm * nrm).sum(1, dtype=F32)).astype(F32)
    nh, nm, nl = _split3(nrm)
    bh, bm, bl = _split3(b)
    rC = np.concatenate(
        [nh.T, nm.T, nh.T, nl.T, nh.T, nm.T,
         bh[None], bm[None], bh[None]], axis=0,
    ).astype(BF)                                                    # [21,V]

    pk = np.zeros((V, 8), F32)
    pk[:, 0:3] = p
    pk[:, 3:6] = nrm
    pk16 = np.ascontiguousarray(pk.reshape(NSEG, SEGW * 8))
    scp = (sc * (1.0 / da)).astype(F32).astype(BF)

    in_maps = []
    for cid in range(N_CORES):
        sl = slice(cid * NQ, (cid + 1) * NQ)
        xc = x[sl]                                                  # [NQ,3]
        xh, xm, xl = _split3(xc)
        x2 = (xc * xc).sum(1, dtype=F32).astype(F32)
        nx2h, nx2m, nx2l = [(-t).astype(F32) for t in _split3(x2)]
        tx2 = [(2.0 * t).astype(F32) for t in (xh, xm, xl)]
        ones_q = np.ones((3, NQ), F32)
        lA = np.concatenate(
            [tx2[0].T, tx2[0].T, tx2[1].T, tx2[0].T, tx2[2].T, tx2[1].T,
             ones_q, nx2h[None], nx2m[None], nx2l[None]], axis=0,
        ).astype(BF)                                                # [24,NQ]
        in_maps.append({
            "lA": lA, "rA": rA, "rC": rC, "pk": pk16, "scp": scp, "xq": xc,
            "idm": np.eye(P, dtype=F32),
        })
    return in_maps, db


def _get_runner(nc):
    """Build (once) a jitted 8-core SPMD runner for the program."""
    import jax
    from jax.sharding import Mesh, PartitionSpec, NamedSharding
    try:
        from jax.experimental.shard_map import shard_map
    except Exception:
        from jax.shard_map import shard_map
    from concourse import bass2jax
    import concourse.mybir as mybir

    bass2jax.install_neuronx_cc_hook()
    pname = nc.partition_id_tensor.name if nc.partition_id_tensor else None
    in_names, out_names, out_avals, zero_outs = [], [], [], []
    for alloc in nc.m.functions[0].allocations:
        if not isinstance(alloc, mybir.MemoryLocationSet):
            continue
        name = alloc.memorylocations[0].name
        if alloc.kind == "ExternalInput":
            if name != pname:
                in_names.append(name)
        elif alloc.kind == "ExternalOutput":
            shape = tuple(alloc.tensor_shape)
            dtype = mybir.dt.np(alloc.dtype)
            out_names.append(name)
            out_avals.append(jax.core.ShapedArray(shape, dtype))
            zero_outs.append(np.zeros(shape, dtype))
    all_names = list(in_names) + list(out_names) + ([pname] if pname else [])

    def _body(*args):
        operands = list(args)
        if pname:
            operands.append(bass2jax.partition_id_tensor())
        return tuple(bass2jax._bass_exec_p.bind(
            *operands, out_avals=tuple(out_avals), in_names=tuple(all_names),
            out_names=tuple(out_names), lowering_input_output_aliases=(),
            sim_require_finite=True, sim_require_nnan=True, nc=nc))

    devices = jax.devices()[:N_CORES]
    mesh = Mesh(np.asarray(devices), ("core",))
    nin = len(in_names) + len(out_names)
    fn = jax.jit(shard_map(
        _body, mesh=mesh, in_specs=(PartitionSpec("core"),) * nin,
        out_specs=(PartitionSpec("core"),) * len(out_names),
        check_rep=False), keep_unused=True)
    sharding = NamedSharding(mesh, PartitionSpec("core"))

    def run(in_maps):
        concat = [np.concatenate([np.asarray(in_maps[c][nm])
                                  for c in range(N_CORES)], axis=0)
                  for nm in in_names]
        concat += [np.concatenate([z] * N_CORES, axis=0) for z in zero_outs]
        import jax as _j
        dev = [_j.device_put(a, sharding) for a in concat]
        outs = fn(*dev)
        o = {nm: np.asarray(outs[i]) for i, nm in enumerate(out_names)}
        return o

    return run


def kernel(**inputs):
    in_maps, db = _prep_inputs(
        inputs["x_world"], inputs["voxel_point"], inputs["voxel_normal"],
        inputs["score"], inputs["d_a"], inputs["d_b"],
    )
    key = ("prog", db)
    if key not in _prog_cache:
        _prog_cache[key] = _build_program(-db)
    nc = _prog_cache[key]

    try:
        rkey = ("runner", db)
        if rkey not in _prog_cache:
            _prog_cache[rkey] = _get_runner(nc)
        o = _prog_cache[rkey](in_maps)
        field = o["of"].reshape(N_CORES, NQ).reshape(-1)
        nzf = o["on"].reshape(-1)
    except Exception:
        from concourse.bass_utils import run_bass_kernel_spmd
        res = run_bass_kernel_spmd(nc, in_maps, list(range(N_CORES))).results
        field = np.concatenate([np.asarray(r["of"]).reshape(-1) for r in res])
        nzf = np.concatenate([np.asarray(r["on"]).reshape(-1) for r in res])
    return field.astype(F32), (nzf > 0.5)



# revision 2
# speedup vs baseline: 2.2779x; 2.2779x over previous
"""Trainium2 Bass kernel for nn_Addparam_25701084299720 (retrieval_knn).

Computes, for N=4096 query points against V=16384 voxels:
  - 8-NN of each query (exact, matching fp32 diff-formulation ordering)
  - mean normal of the 8-NN, cosine-threshold mask vs all voxel normals
  - score_sum = sum_v mask * score_v/d_a * exp(-d_b*dist), score_num = sum mask
  - field = score_sum/max(score_num,1) masked by score_num!=0

Sharding: data-parallel over queries, 512 per core across 8 cores.
Voxel tables replicated.

Per-core algorithm (per 128-query tile):
  A. mmA: psum = 2 x.p - |p|^2 - |x|^2 = -(dist^2), bf16x3-split matmul
     (K=24, ~1e-7 exact). Per chunk: segment-max (16-voxel segments) ->
     segsel[q, 1024] candidate ranking; ACT sqrt psum->D chunk (fp16);
     ACT exp -> E[q, v] (bf16), later scaled in-place by score/d_a (gpsimd).
  B. top-8 segments per half of segsel (DVE max/max_index, 512-wide) ->
     16 segments -> indirect-DMA gather of packed (p, n) segment blocks ->
     exact fp32 diff-formulation rescore of 256 candidates (matches the
     reference's rounding exactly) -> top-8 -> xn = sum of their normals.
  C. mmC: margin = xn.nv - 0.8|xn||nv| (bf16x2/3, K=21). Fused DVE
     scalar_tensor_tensor: ss += sum((margin>0) * E*score'); tensor_scalar:
     cnt += sum(margin>0).
  D. field = ss/max(cnt,1) * (cnt>0).
"""
import sys

sys.path.insert(0, "/opt/trn_rl_repo")

import numpy as np
import ml_dtypes

N_CORES = 8
N = 4096
V = 16384
NQ = N // N_CORES          # 512 queries per core
P = 128                    # partitions
NT = NQ // P               # 4 query tiles per core
VCH = 512                  # voxel chunk (free dim per matmul)
NCH = V // VCH             # 32 chunks
SEGW = 16                  # voxels per segment
NSEG = V // SEGW           # 1024 segments
SPC = VCH // SEGW          # 32 segments per chunk
NCSEG = 16                 # candidate segments (8 per half)
NCAND = NCSEG * SEGW       # 256 candidate voxels
SUBR = 16                  # psum-pair sub-round for ACT batching

BF = ml_dtypes.bfloat16
F32 = np.float32

_prog_cache = {}


def _split3(x):
    """bf16x3 decomposition of fp32 array -> (hi, mid, lo) as float32."""
    hi = x.astype(BF).astype(F32)
    mid = (x - hi).astype(BF).astype(F32)
    lo = (x - hi - mid).astype(BF).astype(F32)
    return hi, mid, lo


def _build_program(neg_db: float, hw: bool = True):
    import concourse.bass as bass
    import concourse.mybir as mybir
    from concourse.tile import TileContext

    nc = bass.Bass()
    dt = mybir.dt
    AF = mybir.ActivationFunctionType
    OP = mybir.AluOpType

    lA_d = nc.declare_dram_parameter("lA", [24, NQ], dt.bfloat16, isOutput=False)
    rA_d = nc.declare_dram_parameter("rA", [24, V], dt.bfloat16, isOutput=False)
    rC_d = nc.declare_dram_parameter("rC", [21, V], dt.bfloat16, isOutput=False)
    pk_d = nc.declare_dram_parameter("pk", [NSEG, SEGW * 8], dt.float32,
                                     isOutput=False)
    scp_d = nc.declare_dram_parameter("scp", [V], dt.bfloat16, isOutput=False)
    xq_d = nc.declare_dram_parameter("xq", [NQ, 3], dt.float32, isOutput=False)
    idm_d = nc.declare_dram_parameter("idm", [P, P], dt.float32, isOutput=False)
    of_d = nc.declare_dram_parameter("of", [NQ], dt.float32, isOutput=True)
    on_d = nc.declare_dram_parameter("on", [NQ], dt.float32, isOutput=True)

    ts = bass.ts
    from concourse.tile_rust import add_dep_helper

    def act(*args, **kwargs):
        return nc.scalar.activation(*args, **kwargs)

    with TileContext(nc) as tc:
        with (
            tc.tile_pool(name="const", bufs=1) as constp,
            tc.tile_pool(name="bigd", bufs=2) as bigp,
            tc.tile_pool(name="small1", bufs=1) as smp1,
            tc.tile_pool(name="chunk", bufs=2) as chp,
            tc.tile_pool(name="dch", bufs=SUBR + 1) as dchp,
            tc.tile_pool(name="small", bufs=2) as smp,
            tc.tile_pool(name="psA", bufs=2, space="PSUM") as psA,
            tc.tile_pool(name="psC", bufs=2, space="PSUM") as psC,
            tc.tile_pool(name="psT", bufs=2, space="PSUM") as psT,
        ):
            lA = constp.tile([24, NQ], dt.bfloat16)
            prc = constp.tile([53, V], dt.bfloat16)
            rA = prc[0:24, :]
            rC = prc[32:53, :]
            scbc = constp.tile([P, V], dt.bfloat16)
            ident = constp.tile([P, P], dt.float32)
            eps4 = constp.tile([P, 1], dt.float32)
            nc.vector.memset(eps4[:], 1e-4)
            nc.sync.dma_start(lA[:], lA_d[:])
            Q4 = V // 4
            nc.sync.dma_start(prc[0:24, 0:Q4], rA_d[:, 0:Q4])
            nc.sync.dma_start(prc[0:24, Q4:2 * Q4], rA_d[:, Q4:2 * Q4])
            nc.scalar.dma_start(prc[0:24, 2 * Q4:3 * Q4], rA_d[:, 2 * Q4:3 * Q4])
            nc.scalar.dma_start(prc[0:24, 3 * Q4:V], rA_d[:, 3 * Q4:V])
            nc.gpsimd.dma_start(prc[32:53, :], rC_d[:])
            nc.gpsimd.dma_start(
                scbc[:],
                scp_d[:].rearrange("(o v) -> o v", o=1).to_broadcast([P, V]),
            )
            nc.gpsimd.dma_start(ident[:], idm_d[:])

            # ---------------- pipelined phase emission ----------------
            def emit_A(i, cprev):
                """Phase A of tile i, with tile i-1's 32 C-chunks interleaved
                (2 per psum-pair) so PE/DVE/Pool keep working while ACT runs
                the sqrt batch."""
                E0 = bigp.tile([P, V // 2], dt.bfloat16, tag="E0")
                E1 = bigp.tile([P, V // 2], dt.bfloat16, tag="E1")
                Eh = [E0, E1]
                segsel = smp.tile([P, NSEG], dt.float16, tag="segsel")
                xqt = smp.tile([P, 3], dt.float32, tag="xqt")
                nc.sync.dma_start(xqt[:], xq_d[ts(i, P), :])
                dchs = []
                sqrt_insts = []
                for u in range(NCH // 2):
                    pA = psA.tile([P, 2 * VCH], dt.float32)
                    for half in range(2):
                        nc.tensor.matmul(
                            pA[:, ts(half, VCH)], lA[:, ts(i, P)],
                            rA[:, ts(2 * u + half, VCH)],
                            start=True, stop=True,
                        )
                    Dch = dchp.tile([P, 2 * VCH], dt.float16, tag="Dch")
                    si = act(
                        Dch[:], pA[:], AF.Sqrt,
                        bias=eps4[:, 0:1], scale=-1.0,
                    )
                    sqrt_insts.append(si)
                    nc.vector.tensor_reduce(
                        segsel[:, ts(u, 2 * SPC)],
                        Dch[:].rearrange("p (s w) -> p s w", w=SEGW),
                        axis=mybir.AxisListType.X, op=OP.min, negate=True,
                    )
                    dchs.append(Dch)
                    if cprev is not None:
                        if u == 2:
                            emit_finishB(cprev)
                        if u >= 3:
                            j0 = 2 * (u - 3)
                            emit_C_chunks(cprev, j0, j0 + 2)
                if cprev is not None:
                    emit_C_chunks(cprev, 26, NCH)
                for k in range(NCH // 2):
                    e_half = Eh[(2 * k) // (NCH // 2)]
                    off = (2 * k * VCH) % (V // 2)
                    ei = act(
                        e_half[:, off:off + 2 * VCH], dchs[k][:],
                        AF.Exp, scale=neg_db,
                    )
                    if k == 0:
                        add_dep_helper(sqrt_insts[-1].ins, ei.ins, sync=True,
                                       reason="exp run after sqrt run")
                # E <- E * score', sliced so it pipelines under the exp run
                NSL = 8
                SL = V // NSL
                for q in range(NSL):
                    h = q // (NSL // 2)
                    off = (q * SL) % (V // 2)
                    nc.gpsimd.tensor_tensor(
                        Eh[h][:, off:off + SL], Eh[h][:, off:off + SL],
                        scbc[:, q * SL:(q + 1) * SL], OP.mult
                    )
                return {"i": i, "Eh": Eh, "segsel": segsel, "xqt": xqt}

            def emit_B(a):
                segsel, xqt = a["segsel"], a["xqt"]
                m8s = smp.tile([P, NCSEG], dt.float16, tag="m8s")
                sidx = smp.tile([P, NCSEG], dt.uint32, tag="sidx")
                HS = NSEG // 2
                nc.vector.max(m8s[:, 0:8], segsel[:, 0:HS])
                nc.vector.max(m8s[:, 8:16], segsel[:, HS:NSEG])
                nc.vector.max_index(sidx[:, 0:8], m8s[:, 0:8], segsel[:, 0:HS])
                nc.vector.max_index(sidx[:, 8:16], m8s[:, 8:16],
                                    segsel[:, HS:NSEG])
                nc.vector.tensor_scalar(
                    sidx[:, 8:16], sidx[:, 8:16], HS, None, OP.add
                )
                pkg = smp1.tile([P, NCSEG, SEGW * 8], dt.float32, tag="pkg")
                # hardware indirect DMA consumes ONE index per partition
                # (tile_scatter_add pattern): one gather per candidate segment
                for g in range(NCSEG):
                    nc.gpsimd.indirect_dma_start(
                        out=pkg[:, g, :], out_offset=None,
                        in_=pk_d[:],
                        in_offset=bass.IndirectOffsetOnAxis(
                            ap=sidx[:, g:g + 1], axis=0),
                    )
                pkv = pkg[:].rearrange("p s (w c) -> p s w c", c=8)
                sq0 = smp1.tile([P, NCAND], dt.float32, tag="sq0")
                sq1 = smp1.tile([P, NCAND], dt.float32, tag="sq1")
                sq2 = smp1.tile([P, NCAND], dt.float32, tag="sq2")
                df0 = smp1.tile([P, NCAND], dt.float32, tag="df0")
                df1 = smp1.tile([P, NCAND], dt.float32, tag="df1")
                df2 = smp1.tile([P, NCAND], dt.float32, tag="df2")
                for c in range(3):
                    df = (df0, df1, df2)[c]
                    sq = (sq0, sq1, sq2)[c]
                    # fl(p - x) then fl(square): same rounding as reference
                    nc.vector.tensor_scalar(
                        df[:], pkv[:, :, :, c], xqt[:, c:c + 1], None,
                        OP.subtract,
                    )
                    nc.gpsimd.tensor_tensor(sq[:], df[:], df[:], OP.mult)
                exd2 = smp1.tile([P, NCAND], dt.float32, tag="exd2")
                nc.gpsimd.tensor_tensor(exd2[:], sq0[:], sq1[:], OP.add)
                nc.gpsimd.tensor_tensor(exd2[:], exd2[:], sq2[:], OP.add)
                negk = smp1.tile([P, NCAND], dt.float32, tag="negk")
                nc.vector.tensor_scalar(negk[:], exd2[:], -1.0, None, OP.mult)
                m8x = smp.tile([P, 8], dt.float32, tag="m8x")
                nc.vector.max(m8x[:], negk[:])
                selx = smp1.tile([P, NCAND], dt.float32, tag="selx")
                nc.vector.tensor_scalar(
                    selx[:], negk[:], m8x[:, 7:8], None, OP.is_ge
                )
                xa4 = smp.tile([P, 4], dt.float32, tag="xa4")
                trc = smp1.tile([P, NCAND], dt.float32, tag="trc")
                for c in range(3):
                    nc.vector.tensor_tensor(
                        trc[:], selx[:], pkv[:, :, :, 3 + c], OP.mult
                    )
                    nc.vector.reduce_sum(
                        xa4[:, c:c + 1], trc[:], axis=mybir.AxisListType.X
                    )
                a2 = smp.tile([P, 1], dt.float32, tag="a2")
                tr3 = smp.tile([P, 3], dt.float32, tag="tr3")
                nc.vector.tensor_tensor(
                    tr3[:], xa4[:, 0:3], xa4[:, 0:3], OP.mult
                )
                nc.vector.reduce_sum(a2[:], tr3[:], axis=mybir.AxisListType.X)


                # lCt [128,21] f32 (free-dim writes only), then PE-transpose.
                # rows k: 0-2 xnh, 3-5 xnh, 6-8 xnm, 9-11 xnh, 12-14 xnl,
                #         15-17 xnm, 18 ch, 19 ch, 20 cm
                lCt = smp.tile([P, 21], dt.float32, tag="lCt")
                bf3 = smp.tile([P, 3], dt.bfloat16, tag="bf3")
                t32 = smp.tile([P, 3], dt.float32, tag="t32")
                rem = smp.tile([P, 3], dt.float32, tag="rem")
                crow = smp.tile([P, 1], dt.float32, tag="crow")
                nc.vector.tensor_copy(bf3[:], xa4[:, 0:3])
                nc.vector.tensor_copy(t32[:], bf3[:])
                nc.vector.tensor_copy(lCt[:, 0:3], t32[:])
                nc.vector.tensor_copy(lCt[:, 3:6], t32[:])
                nc.vector.tensor_copy(lCt[:, 9:12], t32[:])
                nc.vector.tensor_tensor(rem[:], xa4[:, 0:3], t32[:], OP.subtract)
                nc.vector.tensor_copy(bf3[:], rem[:])
                nc.vector.tensor_copy(t32[:], bf3[:])
                nc.vector.tensor_copy(lCt[:, 6:9], t32[:])
                nc.vector.tensor_copy(lCt[:, 15:18], t32[:])
                nc.vector.tensor_tensor(rem[:], rem[:], t32[:], OP.subtract)
                nc.vector.tensor_copy(bf3[:], rem[:])
                nc.vector.tensor_copy(lCt[:, 12:15], bf3[:])
                return {"i": a["i"], "Eh": a["Eh"], "lCt": lCt, "a2": a2,
                        "xa4": xa4, "bf3": bf3, "t32": t32, "rem": rem,
                        "crow": crow}

            def emit_finishB(b):
                xa4, bf3, t32 = b["xa4"], b["bf3"], b["t32"]
                rem, crow, lCt = b["rem"], b["crow"], b["lCt"]
                act(xa4[:, 3:4], b["a2"][:], AF.Sqrt)
                nc.vector.tensor_scalar(
                    crow[:], xa4[:, 3:4], -0.8, None, OP.mult
                )
                nc.vector.tensor_copy(bf3[:, 0:1], crow[:])
                nc.vector.tensor_copy(t32[:, 0:1], bf3[:, 0:1])
                nc.vector.tensor_copy(lCt[:, 18:19], t32[:, 0:1])
                nc.vector.tensor_copy(lCt[:, 19:20], t32[:, 0:1])
                nc.vector.tensor_tensor(
                    rem[:, 0:1], crow[:], t32[:, 0:1], OP.subtract
                )
                nc.vector.tensor_copy(bf3[:, 0:1], rem[:, 0:1])
                nc.vector.tensor_copy(lCt[:, 20:21], bf3[:, 0:1])
                pTC = psT.tile([21, P], dt.float32)
                nc.tensor.transpose(pTC[:], lCt[:], ident[:])
                # lC lives at base partition 32 to match rC (packed in prc)
                lCp = smp.tile([53, P], dt.bfloat16, tag="lCp")
                lC = lCp[32:53, :]
                nc.vector.tensor_copy(lC, pTC[:])
                b["lC"] = lC
                ss32 = smp.tile([P, NCH], dt.float32, tag="ss32")
                cnt32 = smp.tile([P, NCH], dt.float32, tag="cnt32")
                b["ss32"] = ss32
                b["cnt32"] = cnt32

            def emit_C_chunks(b, j0, j1, tail=False):
                lC, Eh, ss32, cnt32 = b["lC"], b["Eh"], b["ss32"], b["cnt32"]
                for j in range(j0, j1):
                    pC = psC.tile([P, VCH], dt.float32)
                    nc.tensor.matmul(
                        pC[:], lC, rC[:, ts(j, VCH)], start=True, stop=True
                    )
                    e_half = Eh[j // (NCH // 2)]
                    off = (j * VCH) % (V // 2)
                    if tail:
                        # ACT is idle in the tail: sign(margin) = +-1, then
                        # ss_j = sum((sign+1)*E) = 2*sum(msk*E),
                        # cnt_j = sum(sign+1)/... = 2*cnt; halved in emit_D.
                        sgn = chp.tile([P, VCH], dt.bfloat16, tag="sgn")
                        act(sgn[:], pC[:], AF.Sign)
                        trb = chp.tile([P, VCH], dt.bfloat16, tag="trb")
                        nc.vector.scalar_tensor_tensor(
                            out=trb[:], in0=sgn[:], scalar=1.0,
                            in1=e_half[:, off:off + VCH],
                            op0=OP.add, op1=OP.mult,
                            accum_out=ss32[:, j:j + 1],
                        )
                        trb2 = chp.tile([P, VCH], dt.bfloat16, tag="trb2")
                        nc.vector.tensor_scalar(
                            trb2[:], pC[:], 0.0, None, OP.is_gt, OP.add,
                            accum_out=cnt32[:, j:j + 1],
                        )
                    else:
                        msk = chp.tile([P, VCH], dt.bfloat16, tag="msk")
                        nc.vector.tensor_scalar(
                            msk[:], pC[:], 0.0, None, OP.is_gt, OP.add,
                            accum_out=cnt32[:, j:j + 1],
                        )
                        ms = chp.tile([P, VCH], dt.bfloat16, tag="ms")
                        nc.gpsimd.tensor_tensor(
                            ms[:], msk[:], e_half[:, off:off + VCH], OP.mult
                        )
                        trb = chp.tile([P, VCH], dt.bfloat16, tag="trb")
                        nc.vector.tensor_scalar(
                            trb[:], ms[:], 0.0, None, OP.add, OP.add,
                            accum_out=ss32[:, j:j + 1],
                        )

            def emit_D(b, tail=False):
                i, ss32, cnt32 = b["i"], b["ss32"], b["cnt32"]
                sst = smp.tile([P, 1], dt.float32, tag="sst")
                cntt = smp.tile([P, 1], dt.float32, tag="cntt")
                nc.vector.reduce_sum(sst[:], ss32[:], axis=mybir.AxisListType.X)
                nc.vector.reduce_sum(cntt[:], cnt32[:],
                                     axis=mybir.AxisListType.X)
                if tail:
                    nc.vector.tensor_scalar(sst[:], sst[:], 0.5, None, OP.mult)
                nz = smp.tile([P, 1], dt.float32, tag="nz")
                nc.vector.tensor_scalar(nz[:], cntt[:], 0.5, None, OP.is_gt)
                cc = smp.tile([P, 1], dt.float32, tag="cc")
                nc.vector.tensor_scalar(cc[:], cntt[:], 1.0, None, OP.max)
                rec = smp.tile([P, 1], dt.float32, tag="rec")
                nc.vector.reciprocal(rec[:], cc[:])
                fld = smp.tile([P, 1], dt.float32, tag="fld")
                nc.vector.tensor_tensor(fld[:], sst[:], rec[:], OP.mult)
                nc.vector.tensor_tensor(fld[:], fld[:], nz[:], OP.mult)
                nc.sync.dma_start(of_d[ts(i, P)], fld[:])
                nc.sync.dma_start(on_d[ts(i, P)], nz[:])

            prev = None
            for i in range(NT):
                a = emit_A(i, prev)
                if prev is not None:
                    emit_D(prev)
                prev = emit_B(a)
            emit_finishB(prev)
            emit_C_chunks(prev, 0, NCH)
            emit_D(prev)

    if hw:
        _split_multiwaits(nc)
    return nc


def _split_multiwaits(nc):
    """This toolchain's walrus accepts at most ONE sync wait per
    instruction (setupSyncWait<...> hard-errors otherwise). Tile attaches
    all required waits to the consuming instruction, so split every
    extra wait into a standalone EventSemaphore on the same engine queue
    right before the instruction (the raw-Bass wait_ge pattern)."""
    import concourse.mybir as mybir

    n = 0
    for bb in nc.main_func.blocks:
        insts = bb.instructions
        out = []
        for inst in insts:
            si = inst.sync_info
            if si is not None and len(si.on_wait) > 1:
                waits = list(si.on_wait)
                for w in waits[:-1]:
                    ev = mybir.InstEventSemaphore(name=f"W-split-{n}")
                    n += 1
                    ev.engine = inst.engine
                    ev.debug = inst.debug
                    ev.sync_info = mybir.SyncInfo(on_wait=[w], on_update=[])
                    out.append(ev)
                inst.sync_info = mybir.SyncInfo(
                    on_wait=[waits[-1]], on_update=list(si.on_update)
                )
            out.append(inst)
        bb.instructions = out


def _prep_inputs(x_world, voxel_point, voxel_normal, score, d_a, d_b):
    """Host-side prep: per-core in_maps for the SPMD program."""
    x = np.ascontiguousarray(x_world[:, 0, :], dtype=F32)          # [N,3]
    p = np.ascontiguousarray(voxel_point[0, :, :3], dtype=F32)     # [V,3]
    nrm = np.ascontiguousarray(voxel_normal, dtype=F32)            # [V,3]
    sc = np.asarray(score, dtype=F32)
    da = float(np.asarray(d_a).reshape(-1)[0])
    db = float(np.asarray(d_b).reshape(-1)[0])

    # rA rows 0-17: p-side of products (xh,ph),(xh,pm),(xm,ph),(xh,pl),
    # (xl,ph),(xm,pm) [3 coord rows each]; 18-20: -p2 h/m/l (lhsT ones);
    # 21-23: ones (lhsT -x2 h/m/l).  psum = 2 x.p - p2 - x2 = -(dist^2)
    ph, pm, pl = _split3(p)
    p2 = (p * p).sum(1, dtype=F32).astype(F32)
    np2h, np2m, np2l = [(-t).astype(F32) for t in _split3(p2)]
    ones_v = np.ones((1, V), F32)
    rA = np.concatenate(
        [ph.T, pm.T, ph.T, pl.T, ph.T, pm.T,
         np2h[None], np2m[None], np2l[None], ones_v, ones_v, ones_v], axis=0,
    ).astype(BF)                                                    # [24,V]

    b = np.sqrt((nrm * nrm).sum(1, dtype=F32)).astype(F32)
    nh, nm, nl = _split3(nrm)
    bh, bm, bl = _split3(b)
    rC = np.concatenate(
        [nh.T, nm.T, nh.T, nl.T, nh.T, nm.T,
         bh[None], bm[None], bh[None]], axis=0,
    ).astype(BF)                                                    # [21,V]

    pk = np.zeros((V, 8), F32)
    pk[:, 0:3] = p
    pk[:, 3:6] = nrm
    pk16 = np.ascontiguousarray(pk.reshape(NSEG, SEGW * 8))
    scp = (sc * (1.0 / da)).astype(F32).astype(BF)

    in_maps = []
    for cid in range(N_CORES):
        sl = slice(cid * NQ, (cid + 1) * NQ)
        xc = x[sl]                                                  # [NQ,3]
        xh, xm, xl = _split3(xc)
        x2 = (xc * xc).sum(1, dtype=F32).astype(F32)
        nx2h, nx2m, nx2l = [(-t).astype(F32) for t in _split3(x2)]
        tx2 = [(2.0 * t).astype(F32) for t in (xh, xm, xl)]
        ones_q = np.ones((3, NQ), F32)
        lA = np.concatenate(
            [tx2[0].T, tx2[0].T, tx2[1].T, tx2[0].T, tx2[2].T, tx2[1].T,
             ones_q, nx2h[None], nx2m[None], nx2l[None]], axis=0,
        ).astype(BF)                                                # [24,NQ]
        in_maps.append({
            "lA": lA, "rA": rA, "rC": rC, "pk": pk16, "scp": scp, "xq": xc,
            "idm": np.eye(P, dtype=F32),
        })
    return in_maps, db


def _get_runner(nc):
    """Build (once) a jitted 8-core SPMD runner for the program."""
    import jax
    from jax.sharding import Mesh, PartitionSpec, NamedSharding
    try:
        from jax.experimental.shard_map import shard_map
    except Exception:
        from jax.shard_map import shard_map
    from concourse import bass2jax
    import concourse.mybir as mybir

    bass2jax.install_neuronx_cc_hook()
    pname = nc.partition_id_tensor.name if nc.partition_id_tensor else None
    in_names, out_names, out_avals, zero_outs = [], [], [], []
    for alloc in nc.m.functions[0].allocations:
        if not isinstance(alloc, mybir.MemoryLocationSet):
            continue
        name = alloc.memorylocations[0].name
        if alloc.kind == "ExternalInput":
            if name != pname:
                in_names.append(name)
        elif alloc.kind == "ExternalOutput":
            shape = tuple(alloc.tensor_shape)
            dtype = mybir.dt.np(alloc.dtype)
            out_names.append(name)
            out_avals.append(jax.core.ShapedArray(shape, dtype))
            zero_outs.append(np.zeros(shape, dtype))
    all_names = list(in_names) + list(out_names) + ([pname] if pname else [])

    def _body(*args):
        operands = list(args)
        if pname:
            operands.append(bass2jax.partition_id_tensor())
        return tuple(bass2jax._bass_exec_p.bind(
            *operands, out_avals=tuple(out_avals), in_names=tuple(all_names),
            out_names=tuple(out_names), lowering_input_output_aliases=(),
            sim_require_finite=True, sim_require_nnan=True, nc=nc))

    devices = jax.devices()[:N_CORES]
    mesh = Mesh(np.asarray(devices), ("core",))
    nin = len(in_names) + len(out_names)

    def make_jit():
        return jax.jit(shard_map(
            _body, mesh=mesh, in_specs=(PartitionSpec("core"),) * nin,
            out_specs=(PartitionSpec("core"),) * len(out_names),
            check_rep=False), keep_unused=True)

    sharding = NamedSharding(mesh, PartitionSpec("core"))
    state = {}

    def run(in_maps):
        concat = [np.concatenate([np.asarray(in_maps[c][nm])
                                  for c in range(N_CORES)], axis=0)
                  for nm in in_names]
        concat += [np.concatenate([z] * N_CORES, axis=0) for z in zero_outs]
        import jax as _j
        dev = [_j.device_put(a, sharding) for a in concat]
        if "fn" not in state:
            # fast-dispatch (C++ path) compile; fall back to plain jit
            try:
                state["fn"] = bass2jax.fast_dispatch_compile(
                    lambda: make_jit().lower(*dev).compile())
            except Exception:
                state["fn"] = make_jit()
        outs = state["fn"](*dev)
        o = {nm: np.asarray(outs[i]) for i, nm in enumerate(out_names)}
        return o

    return run


def kernel(**inputs):
    in_maps, db = _prep_inputs(
        inputs["x_world"], inputs["voxel_point"], inputs["voxel_normal"],
        inputs["score"], inputs["d_a"], inputs["d_b"],
    )
    key = ("prog", db)
    if key not in _prog_cache:
        _prog_cache[key] = _build_program(-db)
    nc = _prog_cache[key]

    try:
        rkey = ("runner", db)
        if rkey not in _prog_cache:
            _prog_cache[rkey] = _get_runner(nc)
        o = _prog_cache[rkey](in_maps)
        field = o["of"].reshape(N_CORES, NQ).reshape(-1)
        nzf = o["on"].reshape(-1)
    except Exception:
        from concourse.bass_utils import run_bass_kernel_spmd
        res = run_bass_kernel_spmd(nc, in_maps, list(range(N_CORES))).results
        field = np.concatenate([np.asarray(r["of"]).reshape(-1) for r in res])
        nzf = np.concatenate([np.asarray(r["on"]).reshape(-1) for r in res])
    return field.astype(F32), (nzf > 0.5)

